# revision 19
# baseline (speedup 1.0000x reference)
"""BiRNN (bidirectional GRU) language model kernel for Trainium2, 8 NeuronCores.

Sharding: data-parallel over batch. Each of the 8 cores takes 2 of the 16 batch
columns and computes everything for its 512 tokens (embedding gather, both GRU
scans, vocab projection, log-softmax) with zero collectives.

Token order per core: t = 2*s + b (s = seq position 0..255, b = local batch 0..1).

Device layout highlights:
  - xT_ext [65, 512]: embedded tokens transposed (E on partitions) + ones row,
    so the gx matmul folds in b_ih.
  - gx precomputed for both directions; r/z part stored in ``gxpre`` (with a
    zero block for the n rows) and PSUM-preloaded before each step's gh matmul
    (start=False accumulate), so the r/z gate adds come free.  xn kept apart.
  - whh_ext [33, 192]: W_hh plus a bias row; h state tiles carry a ones row, so
    the gh matmul folds in b_hh.
  - h' = (1-z)*n + z*h with 1-z computed as sigmoid(-x) on the ACT engine and
    z*h_prev computed while the n-path is still going (both off the critical
    path).
  - h stored into 4 contiguous "shell" tiles [65, 128] (h_l rows 0:32, h_r rows
    32:64, ones row 64); shell k covers seq positions [64k, 64k+64) i.e. token
    rows [128k, 128k+128), so each projection store is one dense 128-partition
    DMA.  Shells are the stationary (lhsT) operand of the vocab projection,
    wout_ext [65, V] carries rnn_out + bias row.
  - log-softmax without a max pass: |logits| <= 65 so exp() cannot overflow
    f32.  Pass 1 computes sum(exp(logits)) per token via Exp+accum_out; pass 2
    recomputes logits and writes logits - log(sum) straight out.
  - wout columns [0, NCACHE) are cached in SBUF (loaded during the scan); the
    remaining columns stream twice (once per pass).
"""

import os
import sys
from contextlib import ExitStack

import numpy as np

for _p in (
    "/opt/trn_rl_repo",
    "/root/.axon_site",
    "/root/.axon_site/_ro/trn_rl_repo",
    "/root/.axon_site/_ro/pypackages",
):
    if os.path.isdir(_p) and _p not in sys.path:
        sys.path.append(_p)

import concourse.bass as bass
import concourse.bacc as bacc
import concourse.tile as tile
from concourse import mybir
from concourse.masks import make_identity

F32 = mybir.dt.float32
BF16 = mybir.dt.bfloat16
I32 = mybir.dt.int32
AF = mybir.ActivationFunctionType
ALU = mybir.AluOpType

V = 50257
E = 64
H = 32
S = 256
B = 16
NCORES = 8
BC = B // NCORES          # batch columns per core
T = S * BC                # tokens per core
G3 = 3 * H                # 96 gate rows
KP = 2 * H + 1            # 65: [h_l; h_r; ones] contraction size for projection
VGRP = 2048               # vocab columns per projection group
NCACHE = 24576            # wout columns cached in SBUF (12 groups)
NGRP_C = NCACHE // VGRP   # cached groups
NGRP_S = (V - NCACHE + VGRP - 1) // VGRP  # streamed groups
NGRP = NGRP_C + NGRP_S


def shell_of(s):
    """Seq position s -> (shell index, column offset).  Shell k holds
    s in [64k, 64k+64), i.e. token rows [128k, 128k+128) of the output."""
    return s // 64, 2 * (s % 64)


def build_module(phases=("pre", "scan", "proj"), use_preload=True):
    # phases may also contain "pass1only" to skip the second projection pass
    nc = bacc.Bacc("TRN2", target_bir_lowering=False)
    tok_h = nc.dram_tensor("tok", (T,), I32, kind="ExternalInput")
    emb_h = nc.dram_tensor("embed", (V, E), F32, kind="ExternalInput")
    wih_h = nc.dram_tensor("wih", (E + 1, 2 * G3), F32, kind="ExternalInput")
    whh_h = nc.dram_tensor("whh", (H + 1, 2 * G3), F32, kind="ExternalInput")
    wout1_h = nc.dram_tensor("wout1", (KP, V), BF16, kind="ExternalInput")
    wout2_h = nc.dram_tensor("wout2", (KP, V), BF16, kind="ExternalInput")
    out_h = nc.dram_tensor("out", (T, V), F32, kind="ExternalOutput")

    with tile.TileContext(nc) as tc:
        with ExitStack() as ctx:
            const = ctx.enter_context(tc.tile_pool(name="const", bufs=1))
            hall = ctx.enter_context(tc.tile_pool(name="hall", bufs=1))

            ident = const.tile([128, 128], F32, tag="ident")
            make_identity(nc, ident[:])
            wih_sb = const.tile([E + 1, 2 * G3], F32, tag="wih")
            nc.sync.dma_start(out=wih_sb[:], in_=wih_h[:])
            whh_sb = const.tile([H + 1, 2 * G3], F32, tag="whh")
            nc.sync.dma_start(out=whh_sb[:], in_=whh_h[:])
            tok_sb = const.tile([128, 4], I32, tag="tok")
            nc.sync.dma_start(out=tok_sb[:], in_=tok_h[:].rearrange("(g p) -> p g", p=128))

            xt = const.tile([E + 1, T], F32, tag="xt")
            nc.vector.memset(xt[E : E + 1, :], 1.0)

            # wout cache for columns [0, NCACHE); DMA issued up front so it
            # overlaps the scan.
            wc1 = hall.tile([KP, NCACHE], BF16, tag="wc1")
            wc2 = hall.tile([KP, NCACHE], BF16, tag="wc2")
            for wc, wh in ((wc1, wout1_h), (wc2, wout2_h)):
                for c0 in range(0, NCACHE, 8192):
                    nc.sync.dma_start(
                        out=wc[:, c0 : c0 + 8192], in_=wh[:][:, c0 : c0 + 8192]
                    )

            hsh = []
            for k in range(4):
                hs = hall.tile([KP, 128], F32, tag=f"hs{k}", name=f"hs{k}")
                nc.vector.memset(hs[2 * H : 2 * H + 1, :], 1.0)
                hsh.append(hs)

            # ping-pong compact GRU state [h; ones] x (L b0, L b1, R b0, R b1)
            hA = const.tile([H + 1, 4], F32, tag="hA")
            hB = const.tile([H + 1, 4], F32, tag="hB")
            nc.vector.memset(hA[:], 0.0)
            nc.vector.memset(hA[H : H + 1, :], 1.0)
            nc.vector.memset(hB[H : H + 1, :], 1.0)

            with (
                tc.tile_pool(name="gath", bufs=2) as gpool,
                tc.tile_pool(name="gx", bufs=1) as gxpool,
                tc.tile_pool(name="scan", bufs=3) as scanp,
                tc.tile_pool(name="ps", bufs=2, space="PSUM") as pspool,
                tc.tile_pool(name="ghp", bufs=3, space="PSUM") as ghpool,
            ):
                # ---- embedding gather + transpose to [E, tokens] ----
                for g in range(4):
                    xg = gpool.tile([128, E], F32, tag="xg")
                    nc.gpsimd.indirect_dma_start(
                        out=xg[:],
                        out_offset=None,
                        in_=emb_h[:],
                        in_offset=bass.IndirectOffsetOnAxis(ap=tok_sb[:, g : g + 1], axis=0),
                    )
                    xps = pspool.tile([E, 128], F32, tag="ps")
                    nc.tensor.transpose(xps[:], xg[:], ident[:])
                    nc.scalar.copy(out=xt[0:E, g * 128 : (g + 1) * 128], in_=xps[:])

                # ---- gx precompute for both directions ----
                # gxpre rows 0:64 = r/z-gate gx (PSUM preload); rows 64:96 zero.
                # xn_all = n-gate gx, added after r*hn.
                # Direction R is stored time-reversed so step t reads column t.
                gxpre = gxpool.tile([G3, S, 4], F32, tag="gxpre")
                xn_all = gxpool.tile([H, S, 4], F32, tag="xnall")
                nc.vector.memset(gxpre[2 * H : G3, :, :], 0.0)
                for d in range(2):
                    gps = pspool.tile([G3, T], F32, tag="ps")
                    nc.tensor.matmul(
                        gps[:], wih_sb[:, d * G3 : (d + 1) * G3], xt[:], start=True, stop=True
                    )
                    if d == 0:
                        src_rz = gps[0 : 2 * H, :].rearrange("p (s b) -> p s b", b=2)
                        src_n = gps[2 * H : G3, :].rearrange("p (s b) -> p s b", b=2)
                    else:
                        base_rz = gps[0 : 2 * H, :]
                        src_rz = bass.AP(
                            tensor=base_rz.tensor,
                            offset=base_rz.offset + (T - 2),
                            ap=[list(base_rz.ap[0]), [-2, S], [1, 2]],
                        )
                        base_n = gps[2 * H : G3, :]
                        src_n = bass.AP(
                            tensor=base_n.tensor,
                            offset=base_n.offset + (T - 2),
                            ap=[list(base_n.ap[0]), [-2, S], [1, 2]],
                        )
                    nc.vector.tensor_copy(out=gxpre[0 : 2 * H, :, 2 * d : 2 * d + 2], in_=src_rz)
                    nc.vector.tensor_copy(out=xn_all[:, :, 2 * d : 2 * d + 2], in_=src_n)

                # ---- the two GRU scans, fused: L at step t, R at step 255-t ----
                for t in range(S if "scan" in phases else 0):
                    sL = t
                    sR = S - 1 - t
                    hp = hA if t % 2 == 0 else hB
                    hn = hB if t % 2 == 0 else hA
                    gh = ghpool.tile([G3, 4], F32, tag="gh")
                    if use_preload:
                        nc.vector.tensor_copy(out=gh[:], in_=gxpre[:, t, :])
                    nc.tensor.matmul(
                        gh[:, 0:2], whh_sb[:, 0:G3], hp[:, 0:2],
                        start=not use_preload, stop=True, skip_group_check=True,
                    )
                    nc.tensor.matmul(
                        gh[:, 2:4], whh_sb[:, G3 : 2 * G3], hp[:, 2:4],
                        start=not use_preload, stop=True, skip_group_check=True,
                    )
                    # Gates via tanh only (sigmoid(x) = .5 + .5*tanh(x/2)):
                    # keeps the ACT table compatible with projection Exp so
                    # pass 1 can overlap the scan tail.
                    rz = scanp.tile([2 * H, 4], F32, tag="rz")
                    nc.scalar.activation(
                        out=rz[:], in_=gh[0 : 2 * H, :], func=AF.Tanh, scale=0.5
                    )
                    # (1-z) = .5 - .5*tz, on Pool, off the critical n path
                    cz = scanp.tile([H, 4], F32, tag="cz")
                    nc.gpsimd.tensor_scalar(cz[:], rz[H : 2 * H, :], -0.5, 0.5,
                                            ALU.mult, ALU.add)
                    # n path: r*hn = .5*(tr+1)*hn, via two fused ops
                    nn = scanp.tile([H, 4], F32, tag="nn")
                    nc.vector.scalar_tensor_tensor(
                        out=nn[:], in0=rz[0:H, :], scalar=1.0, in1=gh[2 * H : G3, :],
                        op0=ALU.add, op1=ALU.mult,
                    )
                    nc.vector.scalar_tensor_tensor(
                        out=nn[:], in0=nn[:], scalar=0.5, in1=xn_all[:, t, :],
                        op0=ALU.mult, op1=ALU.add,
                    )
                    nc.scalar.activation(out=nn[:], in_=nn[:], func=AF.Tanh)
                    # h' = h + (1-z)*(n - h)  ==  z*h + (1-z)*n
                    nc.vector.tensor_sub(nn[:], nn[:], hp[0:H, :])
                    nc.vector.tensor_mul(nn[:], nn[:], cz[:])
                    nc.vector.tensor_add(hn[0:H, :], nn[:], hp[0:H, :])
                    kL, cL = shell_of(sL)
                    kR, cR = shell_of(sR)
                    nc.gpsimd.tensor_copy(out=hsh[kL][0:H, cL : cL + 2], in_=hn[0:H, 0:2])
                    nc.gpsimd.tensor_copy(
                        out=hsh[kR][H : 2 * H, cR : cR + 2], in_=hn[0:H, 2:4]
                    )

            do_proj = "proj" in phases
            if not do_proj and "scan" not in phases:
                for k in range(4):
                    nc.vector.memset(hsh[k][0 : 2 * H, :], 0.0)

            # Split shells into bf16 hi/lo pairs: logits are computed as
            # h1@W1 + h1@W2 + h2@W1 (bf16 matmuls run 4x faster than f32;
            # the dropped h2@W2 term is ~2^-18 relative).
            hs1, hs2 = [], []
            for k in range(4):
                a = hall.tile([KP, 128], BF16, tag=f"hs1_{k}", name=f"hs1_{k}")
                nc.vector.tensor_copy(out=a[:], in_=hsh[k][:])
                b = hall.tile([KP, 128], BF16, tag=f"hs2_{k}", name=f"hs2_{k}")
                nc.vector.tensor_sub(b[:], hsh[k][:], a[:])
                hs1.append(a)
                hs2.append(b)

            # Scheduler-only fence: keeps projection Exp activations from
            # being interleaved with scan Sigmoid/Tanh in the ACT stream
            # (each mix would reload the 1.3us activation table), while DMA
            # prefetches can still run during the scan.
            if do_proj and os.environ.get("KBAR", "1") == "1":
                tc.no_sync_barrier()

            # ---- vocab projection + log-softmax, two passes over wout ----
            with (
                tc.tile_pool(name="wout", bufs=3) as wpool,
                tc.tile_pool(name="outp", bufs=3) as opool,
                tc.tile_pool(name="pp", bufs=2, space="PSUM") as pppool,
            ):
                stats = [
                    const.tile([128, NGRP], F32, tag=f"st{k}", name=f"stats{k}")
                    for k in range(4)
                ]
                negc = [
                    const.tile([128, 1], F32, tag=f"ng{k}", name=f"negc{k}")
                    for k in range(4)
                ]

                def groups(tag):
                    """Yield (group idx, col start, width, (w1, w2) tiles, rhs col0)."""
                    for g in range(NGRP_C):
                        c0 = g * VGRP
                        yield g, c0, VGRP, (wc1, wc2), c0
                    for i in range(NGRP_S):
                        c0 = NCACHE + i * VGRP
                        gw = min(VGRP, V - c0)
                        g = NGRP_C + i
                        wt1 = wpool.tile([KP, VGRP], BF16, tag="wt1", name=f"wt1_{tag}{g}")
                        nc.sync.dma_start(out=wt1[:, 0:gw], in_=wout1_h[:][:, c0 : c0 + gw])
                        wt2 = wpool.tile([KP, VGRP], BF16, tag="wt2", name=f"wt2_{tag}{g}")
                        nc.sync.dma_start(out=wt2[:, 0:gw], in_=wout2_h[:][:, c0 : c0 + gw])
                        yield g, c0, gw, (wt1, wt2), 0

                def emit_pass(tag, finalize):
                    for g, c0, gw, (w1, w2), w0 in groups(tag):
                        for k in range(4):
                            ps = pppool.tile(
                                [128, VGRP], F32, tag="pp", name=f"pp_{tag}{g}_{k}"
                            )
                            for q0 in range(0, gw, 512):
                                qw = min(512, gw - q0)
                                sl = slice(w0 + q0, w0 + q0 + qw)
                                nc.tensor.matmul(
                                    ps[:, q0 : q0 + qw], hs1[k][:], w1[:, sl],
                                    start=True, stop=False,
                                )
                                nc.tensor.matmul(
                                    ps[:, q0 : q0 + qw], hs1[k][:], w2[:, sl],
                                    start=False, stop=False,
                                )
                                nc.tensor.matmul(
                                    ps[:, q0 : q0 + qw], hs2[k][:], w1[:, sl],
                                    start=False, stop=True,
                                )
                            finalize(g, c0, gw, k, ps)

                def fin1(g, c0, gw, k, ps):
                    nc.scalar.activation(
                        out=ps[:, 0:gw], in_=ps[:, 0:gw], func=AF.Exp,
                        accum_out=stats[k][:, g : g + 1],
                    )

                if do_proj:
                    emit_pass("a", fin1)

                for k in range(4 if do_proj else 0):
                    ssum = const.tile([128, 1], F32, tag=f"ss{k}", name=f"ssum{k}")
                    nc.vector.tensor_reduce(
                        out=ssum[:], in_=stats[k][:], axis=mybir.AxisListType.X, op=ALU.add
                    )
                    nc.scalar.activation(out=negc[k][:], in_=ssum[:], func=AF.Ln)
                    nc.vector.tensor_scalar_mul(negc[k][:], negc[k][:], -1.0)

                def fin2(g, c0, gw, k, ps):
                    ob = opool.tile([128, VGRP], F32, tag="ob", name=f"ob{g}_{k}")
                    nc.vector.tensor_scalar_add(ob[:, 0:gw], ps[:, 0:gw], negc[k][:, 0:1])
                    out_base = out_h[:]
                    dst = bass.AP(
                        tensor=out_base.tensor,
                        offset=(128 * k) * V + c0,
                        ap=[[V, 128], [1, gw]],
                    )
                    nc.sync.dma_start(out=dst, in_=ob[:, 0:gw])

                if do_proj and "pass1only" not in phases:
                    emit_pass("b", fin2)
    nc.compile()
    return nc


_CACHE = {}


def _get_module():
    if "nc" not in _CACHE:
        _CACHE["nc"] = build_module()
    return _CACHE["nc"]


def prep_inputs(inputs):
    """Host-side prep: build per-core input maps from the full input dict."""
    ib = np.asarray(inputs["input_batch"])
    embed = np.ascontiguousarray(np.asarray(inputs["embed"], dtype=np.float32))
    rnn_out = np.asarray(inputs["rnn_out"], dtype=np.float32)
    rnn_out_bias = np.asarray(inputs["rnn_out_bias"], dtype=np.float32)

    wih = np.zeros((E + 1, 2 * G3), np.float32)
    wih[:E, :G3] = np.asarray(inputs["Wl_ih"], dtype=np.float32)
    wih[E, :G3] = np.asarray(inputs["bl_ih"], dtype=np.float32)
    wih[:E, G3:] = np.asarray(inputs["Wr_ih"], dtype=np.float32)
    wih[E, G3:] = np.asarray(inputs["br_ih"], dtype=np.float32)

    whh = np.zeros((H + 1, 2 * G3), np.float32)
    whh[:H, :G3] = np.asarray(inputs["Wl_hh"], dtype=np.float32)
    whh[H, :G3] = np.asarray(inputs["bl_hh"], dtype=np.float32)
    whh[:H, G3:] = np.asarray(inputs["Wr_hh"], dtype=np.float32)
    whh[H, G3:] = np.asarray(inputs["br_hh"], dtype=np.float32)

    import ml_dtypes

    wout = np.zeros((KP, V), np.float32)
    wout[0 : 2 * H] = rnn_out
    wout[2 * H] = rnn_out_bias[0]
    wout1 = wout.astype(ml_dtypes.bfloat16)
    wout2 = (wout - wout1.astype(np.float32)).astype(ml_dtypes.bfloat16)

    in_maps = []
    for c in range(NCORES):
        tok = np.ascontiguousarray(
            ib[:, BC * c : BC * (c + 1)].astype(np.int32).reshape(T)
        )
        in_maps.append(
            {"tok": tok, "embed": embed, "wih": wih, "whh": whh,
             "wout1": wout1, "wout2": wout2}
        )
    return in_maps


def assemble_output(results):
    out = np.empty((S, B, V), np.float32)
    for c in range(NCORES):
        out[:, BC * c : BC * (c + 1), :] = results[c]["out"].reshape(S, BC, V)
    return out


def kernel(**inputs):
    from concourse.bass_utils import run_bass_kernel_spmd

    nc = _get_module()
    in_maps = prep_inputs(inputs)
    res = run_bass_kernel_spmd(nc, in_maps, core_ids=list(range(NCORES)))
    return assemble_output(res.results)
